# revision 1
# baseline (speedup 1.0000x reference)
"""
MultiHeadLatentMoE layer as a Bass/Tile kernel for 8 Trainium2 NeuronCores.

Problem (T=8192, D=1024, NH=8 heads, DH=128, NE=8 experts/head, top-2, DHID=512):
    h      = (x @ in_w.T + in_b).reshape(T, NH, DH)
    logits = einsum('tnd,ned->tne', h, router_w)            (fp32)
    gate   = scatter(softmax(top2(logits)))                  (T, NH, NE)
    hid    = gelu(einsum('tnd,nefd->tnef', h, w_in))         (exact erf gelu)
    ye     = einsum('tnef,nefd->tned', hid, w_out)
    y      = einsum('tne,tned->tnd', gate, ye)
    out    = y.reshape(T, NH*DH) @ out_w.T + out_b

Sharding: pure data-parallel over tokens (1024 tokens/core, all heads+experts
local) -> zero collectives.  Per-core output shard is (D, T_loc) transposed;
host concatenates.

Numerics: the big matmuls run as float32r (FP22-truncated reads, full-speed PE).
The in-projection runs as a 3-term hi/lo split (x_hi@w_hi + x_hi@w_lo +
x_lo@w_hi, each term exact in FP22) so h is fp32-accurate; the router matmul is
true fp32.  This keeps top-2 expert selection bit-identical to the fp32
reference (verified: 0/65536 routing flips, ~2e-3 rel L2 err overall from the
fp32r expert/out matmuls).

All tensors on device live feature-on-partitions / tokens-on-free.
"""

import sys

for _p in ("/opt/trn_rl_repo", "/root/.axon_site/_ro/trn_rl_repo"):
    if _p not in sys.path:
        sys.path.append(_p)

import numpy as np

import concourse.bass as bass
import concourse.mybir as mybir
import concourse.tile as tile
from concourse import bacc
from concourse.bass_utils import run_bass_kernel_spmd
from concourse.masks import make_identity

T, D, NH, DH, NE, TOPK, DHID = 8192, 1024, 8, 128, 8, 2, 512
NCORES = 8
TLOC = T // NCORES          # 1024 tokens per core
P = 128
KT = D // P                 # 8 contraction k-tiles for D=1024
TT = TLOC // 512            # 2 moving tiles of 512 tokens
NT = TLOC // P              # 8 token tiles of 128 (router/gate)
FT = DHID // P              # 4 f-tiles per expert
F32 = mybir.dt.float32
F32R = mybir.dt.float32r

_CACHED = None
TRACE = False          # set True (e.g. from test.py) to neuron-profile the run
LAST_RESULT = None     # BassKernelResults of the most recent kernel() call


def build_program():
    nc = bacc.Bacc()

    # ---- DRAM parameters (per-core shards supplied via in_maps) ----
    # float32r-typed params are read (FP22-truncated) by the PE only.
    xt_hi = nc.dram_tensor("xt_hi", [D, TLOC], F32R, kind="ExternalInput")
    xt_lo = nc.dram_tensor("xt_lo", [D, TLOC], F32R, kind="ExternalInput")
    inwt_hi = nc.dram_tensor("inwt_hi", [D, D], F32R, kind="ExternalInput")
    inwt_lo = nc.dram_tensor("inwt_lo", [D, D], F32R, kind="ExternalInput")
    router_dne = nc.dram_tensor("router_dne", [DH, NH, NE], F32, kind="ExternalInput")
    w_int = nc.dram_tensor("w_int", [NH, NE, DH, DHID], F32R, kind="ExternalInput")
    w_outt = nc.dram_tensor("w_outt", [NH, NE, DHID, DH], F32R, kind="ExternalInput")
    out_wt = nc.dram_tensor("out_wt", [D, D], F32R, kind="ExternalInput")
    in_b = nc.dram_tensor("in_b", [D], F32, kind="ExternalInput")
    out_b = nc.dram_tensor("out_b", [D], F32, kind="ExternalInput")
    sel_c = nc.dram_tensor("sel_c", [NE, NE, P], F32R, kind="ExternalInput")
    gate_dram = nc.dram_tensor("gate_dram", [NE, NH, TLOC], F32)
    out_t = nc.dram_tensor("out_t", [D, TLOC], F32, kind="ExternalOutput")

    Act = mybir.ActivationFunctionType
    Alu = mybir.AluOpType

    with tile.TileContext(nc) as tc:
        with (
            tc.tile_pool(name="persist", bufs=1) as persist,
            tc.tile_pool(name="work", bufs=2) as work,
            tc.tile_pool(name="psum", bufs=1, space="PSUM") as psum,
        ):
            # ---- persistent SBUF tensors ----
            ident = persist.tile([P, P], F32, tag="ident")
            make_identity(nc, ident)
            h_sb = persist.tile([P, NH, TLOC], F32, tag="h")     # fp32 (router)
            h_r = persist.tile([P, NH, TLOC], F32R, tag="h_r")   # rounded (experts)
            router_sb = persist.tile([P, NH, NE], F32, tag="router")
            inb_sb = persist.tile([P, NH], F32, tag="inb")
            outb_sb = persist.tile([P, KT], F32, tag="outb")
            nc.sync.dma_start(router_sb[:], router_dne[:])
            nc.sync.dma_start(inb_sb[:], in_b[:].rearrange("(n p) -> p n", p=P))
            nc.sync.dma_start(outb_sb[:], out_b[:].rearrange("(m p) -> p m", p=P))

            # ================= Phase 1: in-projection (split-3 fp32r) ========
            with tc.tile_pool(name="xpool", bufs=1) as xpool, \
                 tc.tile_pool(name="iwpool", bufs=2) as iwpool:
                x_hi = xpool.tile([P, KT, TLOC], F32R, tag="x_hi")
                x_lo = xpool.tile([P, KT, TLOC], F32R, tag="x_lo")
                nc.sync.dma_start(x_hi[:], xt_hi[:].rearrange("(kt p) t -> p kt t", p=P))
                nc.sync.dma_start(x_lo[:], xt_lo[:].rearrange("(kt p) t -> p kt t", p=P))

                # token-half OUTER: all heads' h for tokens 0-511 complete at
                # the phase-1 midpoint, so the router/gate pipeline (phase 2)
                # for those token tiles overlaps the second in-projection half.
                for tt in range(TT):
                    for n in range(NH):
                        w_hi = iwpool.tile([P, KT, DH], F32R, tag="w_hi")
                        w_lo = iwpool.tile([P, KT, DH], F32R, tag="w_lo")
                        csl = slice(n * DH, (n + 1) * DH)
                        nc.sync.dma_start(
                            w_hi[:], inwt_hi[:, csl].rearrange("(kt p) d -> p kt d", p=P))
                        nc.sync.dma_start(
                            w_lo[:], inwt_lo[:, csl].rearrange("(kt p) d -> p kt d", p=P))
                        tsl = slice(tt * 512, (tt + 1) * 512)
                        h_ps = psum.tile([P, 512], F32, tag="misc", bufs=2)
                        terms = [(w_hi, x_hi), (w_lo, x_hi), (w_hi, x_lo)]
                        for i, (wv, xv) in enumerate(terms):
                            for kt in range(KT):
                                nc.tensor.matmul(
                                    h_ps[:],
                                    lhsT=wv[:, kt, :],
                                    rhs=xv[:, kt, tsl],
                                    start=(i == 0 and kt == 0),
                                    stop=(i == len(terms) - 1 and kt == KT - 1),
                                )
                        # h = psum + in_b   (PSUM -> SBUF with per-partition bias)
                        nc.scalar.activation(
                            h_sb[:, n, tsl], h_ps[:], Act.Identity,
                            bias=inb_sb[:, n:n + 1])
                        nc.scalar.activation(
                            h_r[:, n, tsl], h_ps[:], Act.Identity,
                            bias=inb_sb[:, n:n + 1])

            # ---- SBUF for phases 2-4 (allocated after the x pools free) ----
            with tc.tile_pool(name="midpool", bufs=1) as midpool:
                y_sb = midpool.tile([P, NH, TLOC], F32R, tag="y")   # y_nT per head
                # gate, experts-on-partitions: gate_t8[e, n, t]
                gate_t8 = midpool.tile([NE, NH, TLOC], F32, tag="gate_t8")
                # sel[:, e, :]: (NE, P) one-hot row-selector: sel[p,e,c]=1 iff p==e
                sel = midpool.tile([NE, NE, P], F32R, tag="sel")
                nc.sync.dma_start(sel[:], sel_c[:])

                # ================= Phase 2: router + top-2 gate ==================
                for tk in range(NT):
                    tsl = slice(tk * P, (tk + 1) * P)
                    lg_ps = psum.tile([P, NH, NE], F32, tag="misc", bufs=2)
                    for n in range(NH):
                        # logits(t, e) = h(d, t).T @ router(d, e)    true fp32
                        nc.tensor.matmul(
                            lg_ps[:, n, :],
                            lhsT=h_sb[:, n, tsl],
                            rhs=router_sb[:, n, :],
                            start=(n == 0),
                            stop=(n == NH - 1),
                            skip_group_check=True,
                        )
                    lg = work.tile([P, NH, NE], F32, tag="lg")
                    nc.vector.tensor_copy(lg[:], lg_ps[:])
                    m1 = work.tile([P, NH], F32, tag="m1")
                    nc.vector.tensor_reduce(m1[:], lg[:], mybir.AxisListType.X, Alu.max)
                    eq1 = work.tile([P, NH, NE], F32, tag="eq1")
                    nc.vector.tensor_tensor(
                        eq1[:], lg[:], m1[:, :, None].to_broadcast([P, NH, NE]),
                        Alu.is_equal)
                    msk = work.tile([P, NH, NE], F32, tag="msk")
                    # msk = lg - 1e30*eq1
                    nc.vector.scalar_tensor_tensor(
                        msk[:], eq1[:], -1e30, lg[:], Alu.mult, Alu.add)
                    m2 = work.tile([P, NH], F32, tag="m2")
                    nc.vector.tensor_reduce(m2[:], msk[:], mybir.AxisListType.X, Alu.max)
                    eq2 = work.tile([P, NH, NE], F32, tag="eq2")
                    nc.vector.tensor_tensor(
                        eq2[:], lg[:], m2[:, :, None].to_broadcast([P, NH, NE]),
                        Alu.is_equal)
                    dm = work.tile([P, NH], F32, tag="dm")
                    nc.vector.tensor_sub(dm[:], m2[:], m1[:])
                    w2 = work.tile([P, NH], F32, tag="w2")
                    nc.scalar.activation(w2[:], dm[:], Act.Sigmoid)     # w2 = sig(m2-m1)
                    w1 = work.tile([P, NH], F32, tag="w1")
                    nc.vector.tensor_scalar(w1[:], w2[:], -1.0, 1.0, Alu.mult, Alu.add)
                    g1 = work.tile([P, NH, NE], F32, tag="g1")
                    nc.vector.tensor_tensor(
                        g1[:], eq1[:], w1[:, :, None].to_broadcast([P, NH, NE]), Alu.mult)
                    g2 = work.tile([P, NH, NE], F32, tag="g2")
                    nc.vector.tensor_tensor(
                        g2[:], eq2[:], w2[:, :, None].to_broadcast([P, NH, NE]), Alu.mult)
                    gk = work.tile([P, NH * NE], F32, tag="gk")
                    nc.vector.tensor_tensor(
                        gk[:].rearrange("p (n e) -> p n e", n=NH), g1[:], g2[:], Alu.add)
                    # per-head transpose (128t, 8e) -> (8e, 128t)
                    for n in range(NH):
                        tp_ps = psum.tile([NE, P], F32, tag="misc", bufs=2)
                        nc.tensor.transpose(
                            tp_ps[:], gk[:, n * NE:(n + 1) * NE], ident[:])
                        nc.vector.tensor_copy(gate_t8[:, n, tsl], tp_ps[:])

                # stage the full gate to DRAM once (feeds DMA broadcasts)
                nc.sync.dma_start(gate_dram[:], gate_t8[:])

                # ================= Phase 3: experts (dense, fp32r) ===============
                with tc.tile_pool(name="epool", bufs=3) as epool, \
                     tc.tile_pool(name="gpool", bufs=3) as gpool:
                    for n in range(NH):
                        y_ps = psum.tile([P, TT, 512], F32, tag="y", bufs=1)
                        for e in range(NE):
                            wi = epool.tile([P, DHID], F32R, tag="wi")
                            wo = epool.tile([P, FT, DH], F32R, tag="wo")
                            nc.sync.dma_start(wi[:], w_int[n, e])
                            nc.sync.dma_start(
                                wo[:], w_outt[n, e].rearrange("(kt p) d -> p kt d", p=P))
                            gbc_sb = gpool.tile([P, TLOC], F32, tag="gbc_sb")
                            nc.sync.dma_start(
                                gbc_sb[:],
                                gate_dram[e, n][None, :].to_broadcast([P, TLOC]))
                            for tt in range(TT):
                                tsl = slice(tt * 512, (tt + 1) * 512)
                                # two independently-buffered half-experts
                                # (f 0-1 / f 2-3): PE fills half B while
                                # ACT/DVE drain half A
                                for hf in range(2):
                                    hid_ps = psum.tile(
                                        [P, 2, 512], F32, tag=f"hid{hf}", bufs=1)
                                    for fi in range(2):
                                        f = hf * 2 + fi
                                        nc.tensor.matmul(
                                            hid_ps[:, fi, :],
                                            lhsT=wi[:, f * P:(f + 1) * P],
                                            rhs=h_r[:, n, tsl],
                                            start=True, stop=True,
                                        )
                                    hidg = gpool.tile([P, 2, 512], F32, tag="hidg")
                                    nc.scalar.activation(hidg[:], hid_ps[:], Act.Gelu)
                                    hidg_r = gpool.tile([P, 2, 512], F32R, tag="hidg_r")
                                    nc.vector.tensor_tensor(
                                        hidg_r[:], hidg[:],
                                        gbc_sb[:, tsl][:, None, :].to_broadcast(
                                            [P, 2, 512]),
                                        Alu.mult)
                                    for kt in range(2):
                                        nc.tensor.matmul(
                                            y_ps[:, tt, :],
                                            lhsT=wo[:, hf * 2 + kt, :],
                                            rhs=hidg_r[:, kt, :],
                                            start=(e == 0 and hf == 0 and kt == 0),
                                            stop=(e == NE - 1 and hf == 1 and kt == 1),
                                        )
                        nc.vector.tensor_copy(
                            y_sb[:, n, :], y_ps[:].rearrange("p a b -> p (a b)"))

                # ================= Phase 4: out-projection (fp32r) ===============
                with tc.tile_pool(name="opool", bufs=2) as opool:
                    for m in range(KT):
                        ow = opool.tile([P, KT, P], F32R, tag="ow")
                        nc.sync.dma_start(
                            ow[:],
                            out_wt[:, m * P:(m + 1) * P].rearrange(
                                "(kt p) d -> p kt d", p=P))
                        o_sb = opool.tile([P, TLOC], F32, tag="osb")
                        for tt in range(TT):
                            tsl = slice(tt * 512, (tt + 1) * 512)
                            o_ps = psum.tile([P, 512], F32, tag="misc", bufs=2)
                            for kt in range(KT):
                                nc.tensor.matmul(
                                    o_ps[:],
                                    lhsT=ow[:, kt, :],
                                    rhs=y_sb[:, kt, tsl],
                                    start=(kt == 0),
                                    stop=(kt == KT - 1),
                                )
                            nc.scalar.activation(
                                o_sb[:, tsl], o_ps[:], Act.Identity,
                                bias=outb_sb[:, m:m + 1])
                        nc.sync.dma_start(out_t[m * P:(m + 1) * P, :], o_sb[:])

    nc.compile()
    return nc


def _trunc22(a):
    """FP32 -> FP22 truncation (the read path of float32r matmuls)."""
    return (np.ascontiguousarray(a).view(np.uint32) & np.uint32(0xFFFFE000)).view(
        np.float32)


def _prep(x, in_w, in_b, router_w, w_in, w_out, out_w, out_b):
    """Host-side lossless layout prep + hi/lo split; returns per-core in_maps."""
    x = np.ascontiguousarray(x, dtype=np.float32)
    in_wt = np.ascontiguousarray(in_w.T, dtype=np.float32)           # (D, D)
    inwt_hi = _trunc22(in_wt)
    inwt_lo = np.ascontiguousarray(in_wt - inwt_hi)
    shared = {
        "inwt_hi": inwt_hi,
        "inwt_lo": inwt_lo,
        "router_dne": np.ascontiguousarray(
            router_w.transpose(2, 0, 1), dtype=np.float32),          # (DH, NH, NE)
        "w_int": np.ascontiguousarray(
            w_in.transpose(0, 1, 3, 2), dtype=np.float32),           # (NH,NE,DH,DHID)
        "w_outt": np.ascontiguousarray(w_out, dtype=np.float32),     # (NH,NE,DHID,DH)
        "out_wt": np.ascontiguousarray(out_w.T, dtype=np.float32),   # (D, D)
        "in_b": np.ascontiguousarray(in_b, dtype=np.float32),
        "out_b": np.ascontiguousarray(out_b, dtype=np.float32),
        "sel_c": np.ascontiguousarray(
            np.eye(NE, dtype=np.float32)[:, :, None]
            * np.ones((1, 1, P), np.float32)),
    }
    in_maps = []
    for c in range(NCORES):
        xs = x[c * TLOC:(c + 1) * TLOC]                              # (TLOC, D)
        xt = np.ascontiguousarray(xs.T)                              # (D, TLOC)
        xt_hi = _trunc22(xt)
        xt_lo = np.ascontiguousarray(xt - xt_hi)
        in_maps.append({"xt_hi": xt_hi, "xt_lo": xt_lo, **shared})
    return in_maps


def kernel(**inputs) -> np.ndarray:
    global _CACHED
    if _CACHED is None:
        _CACHED = build_program()
    nc = _CACHED
    in_maps = _prep(
        np.asarray(inputs["x"]), np.asarray(inputs["in_w"]),
        np.asarray(inputs["in_b"]), np.asarray(inputs["router_w"]),
        np.asarray(inputs["w_in"]), np.asarray(inputs["w_out"]),
        np.asarray(inputs["out_w"]), np.asarray(inputs["out_b"]))
    global LAST_RESULT
    res = run_bass_kernel_spmd(
        nc, in_maps, core_ids=list(range(NCORES)), trace=TRACE)
    LAST_RESULT = res
    return np.concatenate(
        [np.ascontiguousarray(res.results[c]["out_t"].T) for c in range(NCORES)],
        axis=0)

